# revision 20
# baseline (speedup 1.0000x reference)
"""Trainium2 Bass kernel for nn_CrossAttention — fp8 DoubleRow edition, v4.

Sharding: spatial. Head h covers image rows 4h..4h+3, so core h computes
its head's convs, attention, 1x1 conv and residual with zero cross-core
communication.

vs the bf16 baseline (277us):
  * Six 3x3 convs as fp8-e4m3 DoubleRow matmuls: host expands x into 9
    per-tap windows; the 36 contraction blocks (9 taps x 4 cin chunks)
    pair into 18 DoubleRow matmuls of 256-contraction.  Weights are
    host-prescaled by 32 (fp8 subnormal avoidance); the 32x factors fold
    into the Exp scale (1/32^2) and the softmax ones column (=32).
  * Q/K convs x-stationary -> transposed [spatial, channel] output
    directly (no PE transposes).  V convs weight-stationary.
  * Attention + 1x1 conv + residual stay bf16.

DMA streaming (per-queue HWDGE bandwidth is the head bottleneck):
  * x / weights split in ck-halves (blocks reordered so js 0-8 touch only
    cin chunks 0-1); the first conv runs ck-outer over 4 live psums so
    compute starts after only the first half-tiles have landed.
  * queues: sync = xe_g halves, w5, outs; scalar = w2a, w4, w0, w3, xrs;
    gpsimd = w2b, xe_l halves, w1, wp.  Conv order matches arrival order.
  * outputs are written co-major with 2KB contiguous rows per partition
    and unsharded on the host (512B rows previously throttled the queue).
"""

import numpy as np
import ml_dtypes

N, C, H, W, NH = 4, 512, 32, 32, 8
P = 128
ROWS = H // NH          # 4 output rows per core
S = ROWS * W            # 128 spatial positions per core (= head dim)
CK = C // P             # 4 channel chunks
N_CORES = 8
WSCALE = 32.0           # conv-weight fp8 prescale (power of two)

# 18 DoubleRow contraction pairs over the 36 (cin-chunk, tap) blocks,
# ordered so js 0..8 only touch cin chunks 0-1 and js 9..17 chunks 2-3
# (enables half-tile streaming of both x and weights).
BLOCKS = (
    [("t", ck, t) for ck in (0, 1) for t in (0, 2, 4, 6)] + [("c", 0, 8)]
    + [("t", ck, t) for ck in (2, 3) for t in (0, 2, 4, 6)] + [("c", 2, 8)]
)
NJ = len(BLOCKS)        # 18
HJ = NJ // 2            # 9 per half
# j-block piece boundaries for DMA splitting: tiny leading pieces so the
# first matmul waits only ~130KB, then 3-block pieces
PB = [0, 1, 2, 3, 6, 9, 12, 15, 18]

_BUILT = {}


def _build_bass():
    import concourse.tile as tile
    import concourse.mybir as mybir
    from concourse import bacc

    f32 = mybir.dt.float32
    bf16 = mybir.dt.bfloat16
    f8 = mybir.dt.float8e4
    AF = mybir.ActivationFunctionType
    DR = mybir.MatmulPerfMode.DoubleRow

    nc = bacc.Bacc("TRN2", target_bir_lowering=False)

    SZJ = 2 * N * S      # xe elements per j-block per partition
    SZW = 2 * C          # w3 elements per j-block per partition
    xel_d = nc.dram_tensor("xel", [P, NJ * SZJ], f8, kind="ExternalInput")
    xeg_d = nc.dram_tensor("xeg", [P, NJ * SZJ], f8, kind="ExternalInput")
    xrl_d = nc.dram_tensor("xrl", [P, CK * N * S], f32, kind="ExternalInput")
    xrg_d = nc.dram_tensor("xrg", [P, CK * N * S], f32, kind="ExternalInput")
    # conv order: 0=Wk1(k_l) 1=Wv1(v_l) 2=Wq2*s(q_g) 3=Wq1*s(q_l) 4=Wk2(k_g) 5=Wv2(v_g)
    w3_d = nc.dram_tensor("w3", [6, P, NJ * SZW], f8, kind="ExternalInput")
    wp_d = nc.dram_tensor("wp", [2, P, 2 * 2 * C], f8, kind="ExternalInput")
    outl_d = nc.dram_tensor("out_l", [P, CK, N * S], f32, kind="ExternalOutput")
    outg_d = nc.dram_tensor("out_g", [P, CK, N * S], f32, kind="ExternalOutput")

    import bisect

    with tile.TileContext(nc) as tc:
        with (
            tc.tile_pool(name="singles", bufs=1) as singles,
            tc.tile_pool(name="wpool", bufs=2) as wpool,
            tc.tile_pool(name="attp", bufs=2) as attp,
            tc.tile_pool(name="small", bufs=8) as small,
            tc.tile_pool(name="ps", bufs=5, space="PSUM") as ps_pool,
            tc.tile_pool(name="psy", bufs=3, space="PSUM") as psy_pool,
        ):
            # ---- piece tiles: j-major blocks split for fine-grained DMA
            # dependencies (first matmul waits only for the first piece)
            def mk_xe_tiles(nm):
                return [singles.tile([P, b1 - b0, 2, N, S], f8,
                                     tag=f"xe_{nm}{k}", name=f"xe_{nm}{k}")
                        for k, (b0, b1) in enumerate(zip(PB[:-1], PB[1:]))]

            def mk_w_tiles():
                # per-piece tags so each conv's piece k reuses the slot of
                # the conv two back (bufs=2), giving DMA backpressure
                return [wpool.tile([P, b1 - b0, 2, C], f8, tag=f"w3_{k}",
                                   name=f"w3_{k}")
                        for k, (b0, b1) in enumerate(zip(PB[:-1], PB[1:]))]

            xg_t = mk_xe_tiles("g")
            xl_t = mk_xe_tiles("l")
            # allocation order == consumption order (slot reuse chain)
            wt = {cv: mk_w_tiles() for cv in (2, 4, 0, 3, 1, 5)}

            def dma_w(eng, cv, k):
                b0, b1 = PB[k], PB[k + 1]
                eng.dma_start(
                    wt[cv][k].rearrange("p a b c -> p (a b c)"),
                    w3_d[cv][:, b0 * SZW:b1 * SZW])

            def dma_x(eng, ts, dram, k):
                b0, b1 = PB[k], PB[k + 1]
                eng.dma_start(
                    ts[k].rearrange("p a b c d -> p (a b c d)"),
                    dram[:, b0 * SZJ:b1 * SZJ])

            # sync: first conv's weights+x interleaved in consumption order
            for k in range(len(PB) - 1):
                dma_w(nc.sync, 2, k)
                dma_x(nc.sync, xg_t, xeg_d, k)
            # scalar: weights for convs 2-4, then residuals
            for cv in (4, 0, 3):
                for k in range(len(PB) - 1):
                    dma_w(nc.scalar, cv, k)
            # gpsimd: x_l, then weights for convs 5-6, then wp
            for k in range(len(PB) - 1):
                dma_x(nc.gpsimd, xl_t, xel_d, k)
            for cv in (1, 5):
                for k in range(len(PB) - 1):
                    dma_w(nc.gpsimd, cv, k)

            # wp fp8 pairs: [P, branch, pr, Ko, C] (ci chunk = 2*pr + Ko)
            wp_sb = singles.tile([P, 2, 2, 2, C], f8, tag="wp", name="wp")
            for b in range(2):
                nc.gpsimd.dma_start(
                    wp_sb[:, b].rearrange("p a b c -> p (a b c)"), wp_d[b])

            xr = {}
            for nm, tr in (("g", xrg_d), ("l", xrl_d)):
                xr[nm] = singles.tile([P, CK, N, S], f32, tag=f"xr_{nm}",
                                      name=f"xr_{nm}")
                nc.scalar.dma_start(
                    xr[nm].rearrange("p a b c -> p (a b c)"), tr[:, :])

            def pidx(j):
                return bisect.bisect_right(PB, j) - 1

            def xe_lhsT(xps, j, n):
                k = pidx(j)
                return xps[k][:, j - PB[k], :, n, :]

            def xe_rhs(xps, j):
                k = pidx(j)
                return xps[k][:, j - PB[k]]

            def wblk(cv, j):
                k = pidx(j)
                return wt[cv][k][:, j - PB[k]]

            def mm18(psum, xps, cv, n):
                """18 DoubleRow matmuls accumulating one x-stationary conv."""
                for j in range(NJ):
                    nc.tensor.matmul(
                        psum,
                        lhsT=xe_lhsT(xps, j, n),
                        rhs=wblk(cv, j),
                        start=(j == 0),
                        stop=(j == NJ - 1),
                        perf_mode=DR,
                    )

            def conv_first(xps, cv, dst):
                """First conv, j-outer over 4 live psums: compute starts
                after just the first x/w pieces have landed."""
                psums = [ps_pool.tile([P, C], f32, tag="ps", name="ps")
                         for _ in range(N)]
                for j in range(NJ):
                    for n in range(N):
                        nc.tensor.matmul(
                            psums[n],
                            lhsT=xe_lhsT(xps, j, n),
                            rhs=wblk(cv, j),
                            start=(j == 0),
                            stop=(j == NJ - 1),
                            perf_mode=DR,
                        )
                for n in range(N):
                    nc.vector.tensor_copy(out=dst[:, n, :], in_=psums[n])

            # deferred tensor-queue work: closures fire between conv MM
            # bursts so their upstream copies are long complete (no
            # head-of-line stalls in the in-order PE queue).
            pending = []

            def drain(k=1):
                for _ in range(k):
                    if pending:
                        pending.pop(0)()

            def conv_t(xps, cv, dst, push=None):
                """One x-stationary conv (all batches), transposed output."""
                for n in range(N):
                    psum = ps_pool.tile([P, C], f32, tag="ps", name="ps")
                    mm18(psum, xps, cv, n)
                    nc.vector.tensor_copy(out=dst[:, n, :], in_=psum)
                    drain()
                    if push is not None:
                        pending.append(push(n))

            def conv_v(xps, cv, vdst, drain_k=1):
                """V 3x3 conv, weight-stationary, normal [c, (n s)] output."""
                for co in range(CK):
                    psum = ps_pool.tile([P, C], f32, tag="ps", name="ps")
                    for j in range(NJ):
                        nc.tensor.matmul(
                            psum,
                            lhsT=wblk(cv, j)[:, :, co * P:(co + 1) * P],
                            rhs=xe_rhs(xps, j),
                            start=(j == 0),
                            stop=(j == NJ - 1),
                            perf_mode=DR,
                        )
                    nc.vector.tensor_copy(
                        out=vdst[:, co, :, 0:S],
                        in_=psum.rearrange("p (n s) -> p n s", n=N),
                    )
                    drain(drain_k)

            def qk_batch(q_sb, k_sb, n, pexp):
                """QK^T + exp for one batch into pexp [ck, cq] (bf16)."""
                for ck in range(CK):
                    psum = ps_pool.tile([P, C], f32, tag="ps", name="ps")
                    nc.tensor.matmul(
                        psum,
                        lhsT=k_sb[:, n, ck * P:(ck + 1) * P],
                        rhs=q_sb[:, n, :],
                        start=True,
                        stop=True,
                    )
                    # q,k carry a 32x factor each -> exp scale 1/1024
                    nc.scalar.activation(
                        out=pexp[:, ck, :], in_=psum, func=AF.Exp,
                        scale=1.0 / (WSCALE * WSCALE),
                    )

            def av_group(pexp, vaug, y_sb, cq, n):
                """One AV accumulation group (4 MMs + normalize), deferrable."""
                def run():
                    psy = psy_pool.tile([P, S + 1], f32, tag="psy",
                                        name="psy")
                    for ck in range(CK):
                        nc.tensor.matmul(
                            psy,
                            lhsT=pexp[:, ck, cq * P:(cq + 1) * P],
                            rhs=vaug[:, ck, n, :],
                            start=(ck == 0),
                            stop=(ck == CK - 1),
                        )
                    rec = small.tile([P, 1], f32, tag="rec", name="rec")
                    nc.vector.reciprocal(rec, psy[:, S:S + 1])
                    # ones col = 1 -> rec = 1/sum(exp); V carries 32x so
                    # y_sb holds 32*y, healthy for fp8.
                    nc.scalar.activation(
                        out=y_sb[:, cq, n, :],
                        in_=psy[:, 0:S],
                        func=AF.Copy,
                        scale=rec,
                    )
                return run

            def attend_av(pexps, vaug, y_sb, defer=False):
                for cq in range(CK):
                    for n in range(N):
                        g = av_group(pexps[n], vaug, y_sb, cq, n)
                        if defer:
                            pending.append(g)
                        else:
                            g()

            def proj_groups(y_sb, wp_idx, xres, out_d, out_tag):
                """1x1 conv (fp8 DoubleRow) + residual add + output DMA, as
                one deferrable closure per output chunk.
                psum = 2048*proj(y); xres is host-prescaled by 2048 and the
                host divides the gathered output by 2048 (exact, pow2)."""
                out_sb = singles.tile([P, CK, N, S], f32, tag=f"out_{out_tag}",
                                      name=f"out_{out_tag}")

                def co_group(co):
                    def run():
                        psum = ps_pool.tile([P, C], f32, tag="ps", name="ps")
                        for pr in range(2):
                            nc.tensor.matmul(
                                psum,
                                lhsT=wp_sb[:, wp_idx, pr, :,
                                           co * P:(co + 1) * P],
                                rhs=y_sb[:, 2 * pr:2 * pr + 2],
                                start=(pr == 0),
                                stop=(pr == 1),
                                perf_mode=DR,
                            )
                        nc.vector.tensor_add(
                            out=out_sb[:, co],
                            in0=psum.rearrange("p (n s) -> p n s", n=N),
                            in1=xres[:, co],
                        )
                        # two half-DMAs on separate queues drain the tail 2x
                        H2 = N * S // 2
                        nc.sync.dma_start(
                            out_d[:, co, 0:H2],
                            out_sb[:, co].rearrange("p a b -> p (a b)")[:, 0:H2])
                        nc.scalar.dma_start(
                            out_d[:, co, H2:2 * H2],
                            out_sb[:, co].rearrange(
                                "p a b -> p (a b)")[:, H2:2 * H2])
                    return run
                return [co_group(co) for co in range(CK)]

            # persistent attention operands ([s, n, c] bf16, 32x-scaled)
            k1 = singles.tile([P, N, C], bf16, tag="k1", name="k1")   # k_l
            q2 = singles.tile([P, N, C], bf16, tag="q2", name="q2")   # q_g
            q1 = singles.tile([P, N, C], bf16, tag="q1", name="q1")   # q_l
            k2 = singles.tile([P, N, C], bf16, tag="k2", name="k2")   # k_g
            vl = singles.tile([P, CK, N, S + 1], bf16, tag="vl", name="vl")
            vg = singles.tile([P, CK, N, S + 1], bf16, tag="vg", name="vg")
            # ones column = 1: denominator unscaled, so y_sb keeps V's 32x
            nc.vector.memset(vl[:, :, :, S:S + 1], 1.0)
            nc.vector.memset(vg[:, :, :, S:S + 1], 1.0)

            y_a = singles.tile([P, CK, N, S], f8, tag="y_a", name="y_a")
            y_b = singles.tile([P, CK, N, S], f8, tag="y_b", name="y_b")

            pexps_a = [attp.tile([P, CK, C], bf16, tag="pexp_a",
                                 name="pexp_a", bufs=4) for _ in range(N)]
            pexps_b = [attp.tile([P, CK, C], bf16, tag="pexp_b",
                                 name="pexp_b", bufs=4) for _ in range(N)]

            # branch A: global queries over local k/v -> out_g
            # branch B: local queries over global k/v -> out_l
            conv_first(xg_t, 2, q2)                             # q_g
            conv_t(xg_t, 4, k2)                                 # k_g
            conv_t(xl_t, 0, k1,                                 # k_l; QK-A
                   push=lambda n: (lambda: qk_batch(q2, k1, n, pexps_a[n])))
            conv_t(xl_t, 3, q1,                                 # q_l; QK-B
                   push=lambda n: (lambda: qk_batch(q1, k2, n, pexps_b[n])))
            conv_v(xl_t, 1, vl)                                 # v_l
            attend_av(pexps_a, vl, y_a, defer=True)             # AV-A deferred
            conv_v(xg_t, 5, vg, drain_k=4)                      # v_g + AV-A
            drain(len(pending))
            # AV-B with proj-A chunks interleaved to hide the normalize
            # latency ahead of proj-B
            avb = [av_group(pexps_b[n], vg, y_b, cq, n)
                   for cq in range(CK) for n in range(N)]
            proja = proj_groups(y_a, 1, xr["g"], outg_d, "g")
            projb = proj_groups(y_b, 0, xr["l"], outl_d, "l")
            for i, g in enumerate(avb):
                g()
                if i >= 9 and i % 2 == 1:
                    proja[(i - 9) // 2]()
            for g in projb:
                g()

    nc.finalize()
    return nc


def _prep_host_inputs(x_l, x_g, Wk1, Wq1, Wv1, Wk2, Wq2, Wv2, Wp1, Wp2,
                      resweight):
    """Build the 8 per-core input maps (numpy) from full inputs."""
    bf = ml_dtypes.bfloat16
    f8 = ml_dtypes.float8_e4m3
    scale = 1.0 / np.sqrt(np.float32(S))
    rw = np.float32(np.asarray(resweight))

    def t3(w, s):
        # [cout, cin, 3, 3] -> [9, cin, cout] fp32, scaled
        w = np.asarray(w, np.float32) * s
        return np.ascontiguousarray(w.transpose(2, 3, 1, 0).reshape(9, C, C))

    weffs = [
        t3(Wk1, WSCALE), t3(Wv1, WSCALE), t3(Wq2, scale * WSCALE),
        t3(Wq1, scale * WSCALE), t3(Wk2, WSCALE), t3(Wv2, WSCALE),
    ]
    # pairs per BLOCKS, j-major [6, P, NJ*2*C]
    w3 = np.empty((6, P, NJ, 2, C), np.float32)
    for cv, weff in enumerate(weffs):
        for j, (kind, a, t) in enumerate(BLOCKS):
            if kind == "t":
                w3[cv, :, j, 0] = weff[t, a * P:(a + 1) * P]
                w3[cv, :, j, 1] = weff[t + 1, a * P:(a + 1) * P]
            else:
                w3[cv, :, j, 0] = weff[t, a * P:(a + 1) * P]
                w3[cv, :, j, 1] = weff[t, (a + 1) * P:(a + 2) * P]
    w3 = np.clip(w3, -240, 240).astype(f8).reshape(6, P, NJ * 2 * C)

    # wp [2, P, 2(pr), 2(Ko), C] fp8: Wp.T * rw * 64 (y carries 32x; the
    # 32*64=2048 product scale is divided out on the host after gather)
    def tp(wpm):
        w = np.asarray(wpm, np.float32)[:, :, 0, 0].T * (rw * 64.0)
        return w.reshape(CK, P, C).transpose(1, 0, 2)        # [P, ck, cout]

    f8c = lambda a: np.clip(a, -240, 240).astype(f8)
    wp = f8c(np.stack([tp(Wp1), tp(Wp2)])).reshape(2, P, 2 * 2 * C)

    xl_p = np.pad(np.asarray(x_l, np.float32), ((0, 0), (0, 0), (1, 1), (1, 1)))
    xg_p = np.pad(np.asarray(x_g, np.float32), ((0, 0), (0, 0), (1, 1), (1, 1)))

    def mk_xe(xp, r0):
        # j-major [P, NJ*2*N*S] fp8: per j-block the pre-paired lhsT rows
        out = np.empty((P, CK, 9, N, S), np.float32)
        for t in range(9):
            ky, kx = t // 3, t % 3
            win = xp[:, :, r0 + ky:r0 + ky + ROWS, kx:kx + W]   # [N,C,4,32]
            out[:, :, t] = win.reshape(N, CK, P, S).transpose(2, 1, 0, 3)
        xj = np.empty((P, NJ, 2, N, S), np.float32)
        for j, (kind, a, t) in enumerate(BLOCKS):
            if kind == "t":
                xj[:, j, 0] = out[:, a, t]
                xj[:, j, 1] = out[:, a, t + 1]
            else:
                xj[:, j, 0] = out[:, a, t]
                xj[:, j, 1] = out[:, a + 1, t]
        return np.clip(xj, -240, 240).astype(f8).reshape(P, NJ * 2 * N * S)

    def mk_xr(x, r0):
        # [P, CK, N, S] f32 center rows (co-major), prescaled by 2048 to
        # match the 32*64 scale of the fp8 proj psum (host divides after)
        win = np.asarray(x, np.float32)[:, :, r0:r0 + ROWS, :] * 2048.0
        return np.ascontiguousarray(
            win.reshape(N, CK, P, S).transpose(2, 1, 0, 3)).reshape(P, -1)

    in_maps = []
    for core in range(N_CORES):
        r0 = core * ROWS
        in_maps.append({
            "xel": mk_xe(xl_p, r0),
            "xeg": mk_xe(xg_p, r0),
            "xrl": mk_xr(x_l, r0),
            "xrg": mk_xr(x_g, r0),
            "w3": w3,
            "wp": wp,
        })
    return in_maps


def kernel(x_l, x_g, Wk1, Wq1, Wv1, Wk2, Wq2, Wv2, Wp1, Wp2, resweight,
           _trace=False):
    from concourse.bass_utils import run_bass_kernel_spmd

    if "nc" not in _BUILT:
        _BUILT["nc"] = _build_bass()
    nc = _BUILT["nc"]

    in_maps = _prep_host_inputs(
        x_l, x_g, Wk1, Wq1, Wv1, Wk2, Wq2, Wv2, Wp1, Wp2, resweight
    )
    res = run_bass_kernel_spmd(
        nc, in_maps, core_ids=list(range(N_CORES)), trace=_trace
    )
    out_l = np.empty((N, C, H, W), np.float32)
    out_g = np.empty((N, C, H, W), np.float32)
    for core in range(N_CORES):
        r0 = core * ROWS
        for nm, full in (("out_l", out_l), ("out_g", out_g)):
            r = res.results[core][nm].reshape(P, CK, N, ROWS, W)
            # [p, co, n, r, c] -> full[n, co*128+p, r0+r, c]; /2048 undoes
            # the fp8 proj scale (exact power of two)
            full[:, :, r0:r0 + ROWS, :] = r.transpose(2, 1, 0, 3, 4).reshape(
                N, C, ROWS, W) * (1.0 / 2048.0)
    if _trace:
        kernel.last_result = res
    return out_l, out_g



# revision 21
# speedup vs baseline: 1.0617x; 1.0617x over previous
"""Trainium2 Bass kernel for nn_CrossAttention — fp8 DoubleRow edition, v4.

Sharding: spatial. Head h covers image rows 4h..4h+3, so core h computes
its head's convs, attention, 1x1 conv and residual with zero cross-core
communication.

vs the bf16 baseline (277us):
  * Six 3x3 convs as fp8-e4m3 DoubleRow matmuls: host expands x into 9
    per-tap windows; the 36 contraction blocks (9 taps x 4 cin chunks)
    pair into 18 DoubleRow matmuls of 256-contraction.  Weights are
    host-prescaled by 32 (fp8 subnormal avoidance); the 32x factors fold
    into the Exp scale (1/32^2) and the softmax ones column (=32).
  * Q/K convs x-stationary -> transposed [spatial, channel] output
    directly (no PE transposes).  V convs weight-stationary.
  * Attention + 1x1 conv + residual stay bf16.

DMA streaming (per-queue HWDGE bandwidth is the head bottleneck):
  * x / weights split in ck-halves (blocks reordered so js 0-8 touch only
    cin chunks 0-1); the first conv runs ck-outer over 4 live psums so
    compute starts after only the first half-tiles have landed.
  * queues: sync = xe_g halves, w5, outs; scalar = w2a, w4, w0, w3, xrs;
    gpsimd = w2b, xe_l halves, w1, wp.  Conv order matches arrival order.
  * outputs are written co-major with 2KB contiguous rows per partition
    and unsharded on the host (512B rows previously throttled the queue).
"""

import numpy as np
import ml_dtypes

N, C, H, W, NH = 4, 512, 32, 32, 8
P = 128
ROWS = H // NH          # 4 output rows per core
S = ROWS * W            # 128 spatial positions per core (= head dim)
CK = C // P             # 4 channel chunks
N_CORES = 8
WSCALE = 32.0           # conv-weight fp8 prescale (power of two)

# 18 DoubleRow contraction pairs over the 36 (cin-chunk, tap) blocks,
# ordered so js 0..8 only touch cin chunks 0-1 and js 9..17 chunks 2-3
# (enables half-tile streaming of both x and weights).
BLOCKS = (
    [("t", ck, t) for ck in (0, 1) for t in (0, 2, 4, 6)] + [("c", 0, 8)]
    + [("t", ck, t) for ck in (2, 3) for t in (0, 2, 4, 6)] + [("c", 2, 8)]
)
NJ = len(BLOCKS)        # 18
HJ = NJ // 2            # 9 per half
# j-block piece boundaries for DMA splitting: tiny leading pieces so the
# first matmul waits only ~130KB, then 3-block pieces
PB = [0, 1, 2, 3, 6, 9, 12, 15, 18]

_BUILT = {}


def _build_bass():
    import concourse.tile as tile
    import concourse.mybir as mybir
    from concourse import bacc

    f32 = mybir.dt.float32
    bf16 = mybir.dt.bfloat16
    f8 = mybir.dt.float8e4
    AF = mybir.ActivationFunctionType
    DR = mybir.MatmulPerfMode.DoubleRow

    nc = bacc.Bacc("TRN2", target_bir_lowering=False)

    SZJ = 2 * N * S      # xe elements per j-block per partition
    SZW = 2 * C          # w3 elements per j-block per partition
    xel_d = nc.dram_tensor("xel", [P, NJ * SZJ], f8, kind="ExternalInput")
    xeg_d = nc.dram_tensor("xeg", [P, NJ * SZJ], f8, kind="ExternalInput")
    xrl_d = nc.dram_tensor("xrl", [P, CK * N * S], f32, kind="ExternalInput")
    xrg_d = nc.dram_tensor("xrg", [P, CK * N * S], f32, kind="ExternalInput")
    # conv order: 0=Wk1(k_l) 1=Wv1(v_l) 2=Wq2*s(q_g) 3=Wq1*s(q_l) 4=Wk2(k_g) 5=Wv2(v_g)
    w3_d = nc.dram_tensor("w3", [6, P, NJ * SZW], f8, kind="ExternalInput")
    wp_d = nc.dram_tensor("wp", [2, P, 2 * 2 * C], f8, kind="ExternalInput")
    outl_d = nc.dram_tensor("out_l", [P, CK, N * S], f32, kind="ExternalOutput")
    outg_d = nc.dram_tensor("out_g", [P, CK, N * S], f32, kind="ExternalOutput")

    import bisect

    with tile.TileContext(nc) as tc:
        with (
            tc.tile_pool(name="singles", bufs=1) as singles,
            tc.tile_pool(name="wpool", bufs=2) as wpool,
            tc.tile_pool(name="attp", bufs=2) as attp,
            tc.tile_pool(name="small", bufs=8) as small,
            tc.tile_pool(name="ps", bufs=5, space="PSUM") as ps_pool,
            tc.tile_pool(name="psy", bufs=3, space="PSUM") as psy_pool,
        ):
            # ---- piece tiles: j-major blocks split for fine-grained DMA
            # dependencies (first matmul waits only for the first piece)
            def mk_xe_tiles(nm):
                return [singles.tile([P, b1 - b0, 2, N, S], f8,
                                     tag=f"xe_{nm}{k}", name=f"xe_{nm}{k}")
                        for k, (b0, b1) in enumerate(zip(PB[:-1], PB[1:]))]

            def mk_w_tiles():
                # per-piece tags so each conv's piece k reuses the slot of
                # the conv two back (bufs=2), giving DMA backpressure
                return [wpool.tile([P, b1 - b0, 2, C], f8, tag=f"w3_{k}",
                                   name=f"w3_{k}")
                        for k, (b0, b1) in enumerate(zip(PB[:-1], PB[1:]))]

            xg_t = mk_xe_tiles("g")
            xl_t = mk_xe_tiles("l")
            # allocation order == consumption order (slot reuse chain)
            wt = {cv: mk_w_tiles() for cv in (2, 4, 0, 3, 1, 5)}

            def dma_w(eng, cv, k):
                b0, b1 = PB[k], PB[k + 1]
                eng.dma_start(
                    wt[cv][k].rearrange("p a b c -> p (a b c)"),
                    w3_d[cv][:, b0 * SZW:b1 * SZW])

            def dma_x(eng, ts, dram, k):
                b0, b1 = PB[k], PB[k + 1]
                eng.dma_start(
                    ts[k].rearrange("p a b c d -> p (a b c d)"),
                    dram[:, b0 * SZJ:b1 * SZJ])

            # conv1's ~300GB/s piece stream is split across the two fast
            # queues (sync starts first and takes the tiny lead pieces);
            # later convs' weights are prefetched on whichever queue frees
            # up in consumption order. gpsimd (slow software queue) only
            # carries late-needed weights.
            NP = len(PB) - 1
            for k in range(NP):
                eng = nc.sync if k in (0, 1, 2, 4, 6) else nc.scalar
                dma_w(eng, 2, k)
                dma_x(eng, xg_t, xeg_d, k)
            for k in range(NP):
                dma_w(nc.sync, 4, k)
            for k in range(NP):
                dma_x(nc.sync, xl_t, xel_d, k)
            for cv in (0, 3):
                for k in range(NP):
                    dma_w(nc.scalar, cv, k)
            for cv in (1, 5):
                for k in range(NP):
                    dma_w(nc.gpsimd, cv, k)

            # wp fp8 pairs: [P, branch, pr, Ko, C] (ci chunk = 2*pr + Ko)
            wp_sb = singles.tile([P, 2, 2, 2, C], f8, tag="wp", name="wp")
            for b in range(2):
                nc.gpsimd.dma_start(
                    wp_sb[:, b].rearrange("p a b c -> p (a b c)"), wp_d[b])

            xr = {}
            for nm, tr in (("g", xrg_d), ("l", xrl_d)):
                xr[nm] = singles.tile([P, CK, N, S], f32, tag=f"xr_{nm}",
                                      name=f"xr_{nm}")
                nc.scalar.dma_start(
                    xr[nm].rearrange("p a b c -> p (a b c)"), tr[:, :])

            def pidx(j):
                return bisect.bisect_right(PB, j) - 1

            def xe_lhsT(xps, j, n):
                k = pidx(j)
                return xps[k][:, j - PB[k], :, n, :]

            def xe_rhs(xps, j):
                k = pidx(j)
                return xps[k][:, j - PB[k]]

            def wblk(cv, j):
                k = pidx(j)
                return wt[cv][k][:, j - PB[k]]

            def mm18(psum, xps, cv, n):
                """18 DoubleRow matmuls accumulating one x-stationary conv."""
                for j in range(NJ):
                    nc.tensor.matmul(
                        psum,
                        lhsT=xe_lhsT(xps, j, n),
                        rhs=wblk(cv, j),
                        start=(j == 0),
                        stop=(j == NJ - 1),
                        perf_mode=DR,
                    )

            def conv_first(xps, cv, dst):
                """First conv, j-outer over 4 live psums: compute starts
                after just the first x/w pieces have landed."""
                psums = [ps_pool.tile([P, C], f32, tag="ps", name="ps")
                         for _ in range(N)]
                for j in range(NJ):
                    for n in range(N):
                        nc.tensor.matmul(
                            psums[n],
                            lhsT=xe_lhsT(xps, j, n),
                            rhs=wblk(cv, j),
                            start=(j == 0),
                            stop=(j == NJ - 1),
                            perf_mode=DR,
                        )
                for n in range(N):
                    nc.vector.tensor_copy(out=dst[:, n, :], in_=psums[n])

            # deferred tensor-queue work: closures fire between conv MM
            # bursts so their upstream copies are long complete (no
            # head-of-line stalls in the in-order PE queue).
            pending = []

            def drain(k=1):
                for _ in range(k):
                    if pending:
                        pending.pop(0)()

            def conv_t(xps, cv, dst, push=None):
                """One x-stationary conv (all batches), transposed output."""
                for n in range(N):
                    psum = ps_pool.tile([P, C], f32, tag="ps", name="ps")
                    mm18(psum, xps, cv, n)
                    nc.vector.tensor_copy(out=dst[:, n, :], in_=psum)
                    drain()
                    if push is not None:
                        pending.append(push(n))

            def conv_v(xps, cv, vdst, drain_k=1):
                """V 3x3 conv, weight-stationary, normal [c, (n s)] output."""
                for co in range(CK):
                    psum = ps_pool.tile([P, C], f32, tag="ps", name="ps")
                    for j in range(NJ):
                        nc.tensor.matmul(
                            psum,
                            lhsT=wblk(cv, j)[:, :, co * P:(co + 1) * P],
                            rhs=xe_rhs(xps, j),
                            start=(j == 0),
                            stop=(j == NJ - 1),
                            perf_mode=DR,
                        )
                    nc.vector.tensor_copy(
                        out=vdst[:, co, :, 0:S],
                        in_=psum.rearrange("p (n s) -> p n s", n=N),
                    )
                    drain(drain_k)

            def qk_batch(q_sb, k_sb, n, pexp):
                """QK^T + exp for one batch into pexp [ck, cq] (bf16)."""
                for ck in range(CK):
                    psum = ps_pool.tile([P, C], f32, tag="ps", name="ps")
                    nc.tensor.matmul(
                        psum,
                        lhsT=k_sb[:, n, ck * P:(ck + 1) * P],
                        rhs=q_sb[:, n, :],
                        start=True,
                        stop=True,
                    )
                    # q,k carry a 32x factor each -> exp scale 1/1024
                    nc.scalar.activation(
                        out=pexp[:, ck, :], in_=psum, func=AF.Exp,
                        scale=1.0 / (WSCALE * WSCALE),
                    )

            def av_group(pexp, vaug, y_sb, cq, n):
                """One AV accumulation group (4 MMs + normalize), deferrable."""
                def run():
                    psy = psy_pool.tile([P, S + 1], f32, tag="psy",
                                        name="psy")
                    for ck in range(CK):
                        nc.tensor.matmul(
                            psy,
                            lhsT=pexp[:, ck, cq * P:(cq + 1) * P],
                            rhs=vaug[:, ck, n, :],
                            start=(ck == 0),
                            stop=(ck == CK - 1),
                        )
                    rec = small.tile([P, 1], f32, tag="rec", name="rec")
                    nc.vector.reciprocal(rec, psy[:, S:S + 1])
                    # ones col = 1 -> rec = 1/sum(exp); V carries 32x so
                    # y_sb holds 32*y, healthy for fp8.
                    nc.scalar.activation(
                        out=y_sb[:, cq, n, :],
                        in_=psy[:, 0:S],
                        func=AF.Copy,
                        scale=rec,
                    )
                return run

            def attend_av(pexps, vaug, y_sb, defer=False):
                for cq in range(CK):
                    for n in range(N):
                        g = av_group(pexps[n], vaug, y_sb, cq, n)
                        if defer:
                            pending.append(g)
                        else:
                            g()

            def proj_groups(y_sb, wp_idx, xres, out_d, out_tag):
                """1x1 conv (fp8 DoubleRow) + residual add + output DMA, as
                one deferrable closure per output chunk.
                psum = 2048*proj(y); xres is host-prescaled by 2048 and the
                host divides the gathered output by 2048 (exact, pow2)."""
                out_sb = singles.tile([P, CK, N, S], f32, tag=f"out_{out_tag}",
                                      name=f"out_{out_tag}")

                def co_group(co):
                    def run():
                        psum = ps_pool.tile([P, C], f32, tag="ps", name="ps")
                        for pr in range(2):
                            nc.tensor.matmul(
                                psum,
                                lhsT=wp_sb[:, wp_idx, pr, :,
                                           co * P:(co + 1) * P],
                                rhs=y_sb[:, 2 * pr:2 * pr + 2],
                                start=(pr == 0),
                                stop=(pr == 1),
                                perf_mode=DR,
                            )
                        nc.vector.tensor_add(
                            out=out_sb[:, co],
                            in0=psum.rearrange("p (n s) -> p n s", n=N),
                            in1=xres[:, co],
                        )
                        # two half-DMAs on separate queues drain the tail 2x
                        H2 = N * S // 2
                        nc.sync.dma_start(
                            out_d[:, co, 0:H2],
                            out_sb[:, co].rearrange("p a b -> p (a b)")[:, 0:H2])
                        nc.scalar.dma_start(
                            out_d[:, co, H2:2 * H2],
                            out_sb[:, co].rearrange(
                                "p a b -> p (a b)")[:, H2:2 * H2])
                    return run
                return [co_group(co) for co in range(CK)]

            # persistent attention operands ([s, n, c] bf16, 32x-scaled)
            k1 = singles.tile([P, N, C], bf16, tag="k1", name="k1")   # k_l
            q2 = singles.tile([P, N, C], bf16, tag="q2", name="q2")   # q_g
            q1 = singles.tile([P, N, C], bf16, tag="q1", name="q1")   # q_l
            k2 = singles.tile([P, N, C], bf16, tag="k2", name="k2")   # k_g
            vl = singles.tile([P, CK, N, S + 1], bf16, tag="vl", name="vl")
            vg = singles.tile([P, CK, N, S + 1], bf16, tag="vg", name="vg")
            # ones column = 1: denominator unscaled, so y_sb keeps V's 32x
            nc.vector.memset(vl[:, :, :, S:S + 1], 1.0)
            nc.vector.memset(vg[:, :, :, S:S + 1], 1.0)

            y_a = singles.tile([P, CK, N, S], f8, tag="y_a", name="y_a")
            y_b = singles.tile([P, CK, N, S], f8, tag="y_b", name="y_b")

            pexps_a = [attp.tile([P, CK, C], bf16, tag="pexp_a",
                                 name="pexp_a", bufs=4) for _ in range(N)]
            pexps_b = [attp.tile([P, CK, C], bf16, tag="pexp_b",
                                 name="pexp_b", bufs=4) for _ in range(N)]

            # branch A: global queries over local k/v -> out_g
            # branch B: local queries over global k/v -> out_l
            conv_first(xg_t, 2, q2)                             # q_g
            conv_t(xg_t, 4, k2)                                 # k_g
            conv_t(xl_t, 0, k1,                                 # k_l; QK-A
                   push=lambda n: (lambda: qk_batch(q2, k1, n, pexps_a[n])))
            conv_t(xl_t, 3, q1,                                 # q_l; QK-B
                   push=lambda n: (lambda: qk_batch(q1, k2, n, pexps_b[n])))
            conv_v(xl_t, 1, vl)                                 # v_l
            attend_av(pexps_a, vl, y_a, defer=True)             # AV-A deferred
            conv_v(xg_t, 5, vg, drain_k=4)                      # v_g + AV-A
            drain(len(pending))
            # AV-B with proj-A chunks interleaved to hide the normalize
            # latency ahead of proj-B
            avb = [av_group(pexps_b[n], vg, y_b, cq, n)
                   for cq in range(CK) for n in range(N)]
            proja = proj_groups(y_a, 1, xr["g"], outg_d, "g")
            projb = proj_groups(y_b, 0, xr["l"], outl_d, "l")
            for i, g in enumerate(avb):
                g()
                if i >= 9 and i % 2 == 1:
                    proja[(i - 9) // 2]()
            for g in projb:
                g()

    nc.finalize()
    return nc


def _prep_host_inputs(x_l, x_g, Wk1, Wq1, Wv1, Wk2, Wq2, Wv2, Wp1, Wp2,
                      resweight):
    """Build the 8 per-core input maps (numpy) from full inputs."""
    bf = ml_dtypes.bfloat16
    f8 = ml_dtypes.float8_e4m3
    scale = 1.0 / np.sqrt(np.float32(S))
    rw = np.float32(np.asarray(resweight))

    def t3(w, s):
        # [cout, cin, 3, 3] -> [9, cin, cout] fp32, scaled
        w = np.asarray(w, np.float32) * s
        return np.ascontiguousarray(w.transpose(2, 3, 1, 0).reshape(9, C, C))

    weffs = [
        t3(Wk1, WSCALE), t3(Wv1, WSCALE), t3(Wq2, scale * WSCALE),
        t3(Wq1, scale * WSCALE), t3(Wk2, WSCALE), t3(Wv2, WSCALE),
    ]
    # pairs per BLOCKS, j-major [6, P, NJ*2*C]
    w3 = np.empty((6, P, NJ, 2, C), np.float32)
    for cv, weff in enumerate(weffs):
        for j, (kind, a, t) in enumerate(BLOCKS):
            if kind == "t":
                w3[cv, :, j, 0] = weff[t, a * P:(a + 1) * P]
                w3[cv, :, j, 1] = weff[t + 1, a * P:(a + 1) * P]
            else:
                w3[cv, :, j, 0] = weff[t, a * P:(a + 1) * P]
                w3[cv, :, j, 1] = weff[t, (a + 1) * P:(a + 2) * P]
    w3 = np.clip(w3, -240, 240).astype(f8).reshape(6, P, NJ * 2 * C)

    # wp [2, P, 2(pr), 2(Ko), C] fp8: Wp.T * rw * 64 (y carries 32x; the
    # 32*64=2048 product scale is divided out on the host after gather)
    def tp(wpm):
        w = np.asarray(wpm, np.float32)[:, :, 0, 0].T * (rw * 64.0)
        return w.reshape(CK, P, C).transpose(1, 0, 2)        # [P, ck, cout]

    f8c = lambda a: np.clip(a, -240, 240).astype(f8)
    wp = f8c(np.stack([tp(Wp1), tp(Wp2)])).reshape(2, P, 2 * 2 * C)

    xl_p = np.pad(np.asarray(x_l, np.float32), ((0, 0), (0, 0), (1, 1), (1, 1)))
    xg_p = np.pad(np.asarray(x_g, np.float32), ((0, 0), (0, 0), (1, 1), (1, 1)))

    def mk_xe(xp, r0):
        # j-major [P, NJ*2*N*S] fp8: per j-block the pre-paired lhsT rows
        out = np.empty((P, CK, 9, N, S), np.float32)
        for t in range(9):
            ky, kx = t // 3, t % 3
            win = xp[:, :, r0 + ky:r0 + ky + ROWS, kx:kx + W]   # [N,C,4,32]
            out[:, :, t] = win.reshape(N, CK, P, S).transpose(2, 1, 0, 3)
        xj = np.empty((P, NJ, 2, N, S), np.float32)
        for j, (kind, a, t) in enumerate(BLOCKS):
            if kind == "t":
                xj[:, j, 0] = out[:, a, t]
                xj[:, j, 1] = out[:, a, t + 1]
            else:
                xj[:, j, 0] = out[:, a, t]
                xj[:, j, 1] = out[:, a + 1, t]
        return np.clip(xj, -240, 240).astype(f8).reshape(P, NJ * 2 * N * S)

    def mk_xr(x, r0):
        # [P, CK, N, S] f32 center rows (co-major), prescaled by 2048 to
        # match the 32*64 scale of the fp8 proj psum (host divides after)
        win = np.asarray(x, np.float32)[:, :, r0:r0 + ROWS, :] * 2048.0
        return np.ascontiguousarray(
            win.reshape(N, CK, P, S).transpose(2, 1, 0, 3)).reshape(P, -1)

    in_maps = []
    for core in range(N_CORES):
        r0 = core * ROWS
        in_maps.append({
            "xel": mk_xe(xl_p, r0),
            "xeg": mk_xe(xg_p, r0),
            "xrl": mk_xr(x_l, r0),
            "xrg": mk_xr(x_g, r0),
            "w3": w3,
            "wp": wp,
        })
    return in_maps


def kernel(x_l, x_g, Wk1, Wq1, Wv1, Wk2, Wq2, Wv2, Wp1, Wp2, resweight,
           _trace=False):
    from concourse.bass_utils import run_bass_kernel_spmd

    if "nc" not in _BUILT:
        _BUILT["nc"] = _build_bass()
    nc = _BUILT["nc"]

    in_maps = _prep_host_inputs(
        x_l, x_g, Wk1, Wq1, Wv1, Wk2, Wq2, Wv2, Wp1, Wp2, resweight
    )
    res = run_bass_kernel_spmd(
        nc, in_maps, core_ids=list(range(N_CORES)), trace=_trace
    )
    out_l = np.empty((N, C, H, W), np.float32)
    out_g = np.empty((N, C, H, W), np.float32)
    for core in range(N_CORES):
        r0 = core * ROWS
        for nm, full in (("out_l", out_l), ("out_g", out_g)):
            r = res.results[core][nm].reshape(P, CK, N, ROWS, W)
            # [p, co, n, r, c] -> full[n, co*128+p, r0+r, c]; /2048 undoes
            # the fp8 proj scale (exact power of two)
            full[:, :, r0:r0 + ROWS, :] = r.transpose(2, 1, 0, 3, 4).reshape(
                N, C, ROWS, W) * (1.0 / 2048.0)
    if _trace:
        kernel.last_result = res
    return out_l, out_g



# revision 23
# speedup vs baseline: 1.2179x; 1.1471x over previous
"""Trainium2 Bass kernel for nn_CrossAttention — fp8 DoubleRow edition, v4.

Sharding: spatial. Head h covers image rows 4h..4h+3, so core h computes
its head's convs, attention, 1x1 conv and residual with zero cross-core
communication.

vs the bf16 baseline (277us):
  * Six 3x3 convs as fp8-e4m3 DoubleRow matmuls: host expands x into 9
    per-tap windows; the 36 contraction blocks (9 taps x 4 cin chunks)
    pair into 18 DoubleRow matmuls of 256-contraction.  Weights are
    host-prescaled by 32 (fp8 subnormal avoidance); the 32x factors fold
    into the Exp scale (1/32^2) and the softmax ones column (=32).
  * Q/K convs x-stationary -> transposed [spatial, channel] output
    directly (no PE transposes).  V convs weight-stationary.
  * Attention + 1x1 conv + residual stay bf16.

DMA streaming (per-queue HWDGE bandwidth is the head bottleneck):
  * x / weights split in ck-halves (blocks reordered so js 0-8 touch only
    cin chunks 0-1); the first conv runs ck-outer over 4 live psums so
    compute starts after only the first half-tiles have landed.
  * queues: sync = xe_g halves, w5, outs; scalar = w2a, w4, w0, w3, xrs;
    gpsimd = w2b, xe_l halves, w1, wp.  Conv order matches arrival order.
  * outputs are written co-major with 2KB contiguous rows per partition
    and unsharded on the host (512B rows previously throttled the queue).
"""

import numpy as np
import ml_dtypes

N, C, H, W, NH = 4, 512, 32, 32, 8
P = 128
ROWS = H // NH          # 4 output rows per core
S = ROWS * W            # 128 spatial positions per core (= head dim)
CK = C // P             # 4 channel chunks
N_CORES = 8
WSCALE = 32.0           # conv-weight fp8 prescale (power of two)

# 18 DoubleRow contraction pairs over the 36 (cin-chunk, tap) blocks,
# ordered so js 0..8 only touch cin chunks 0-1 and js 9..17 chunks 2-3
# (enables half-tile streaming of both x and weights).
BLOCKS = (
    [("t", ck, t) for ck in (0, 1) for t in (0, 2, 4, 6)] + [("c", 0, 8)]
    + [("t", ck, t) for ck in (2, 3) for t in (0, 2, 4, 6)] + [("c", 2, 8)]
)
NJ = len(BLOCKS)        # 18
HJ = NJ // 2            # 9 per half
# j-block piece boundaries for DMA splitting: tiny leading pieces so the
# first matmul waits only ~130KB, then 3-block pieces
PB = [0, 1, 2, 3, 6, 9, 12, 15, 18]

_BUILT = {}


def _build_bass():
    import concourse.tile as tile
    import concourse.mybir as mybir
    from concourse import bacc

    f32 = mybir.dt.float32
    bf16 = mybir.dt.bfloat16
    f8 = mybir.dt.float8e4
    AF = mybir.ActivationFunctionType
    DR = mybir.MatmulPerfMode.DoubleRow

    nc = bacc.Bacc("TRN2", target_bir_lowering=False)

    SZJ = 2 * N * S      # xe elements per j-block per partition
    SZW = 2 * C          # w3 elements per j-block per partition
    xel_d = nc.dram_tensor("xel", [P, NJ * SZJ], f8, kind="ExternalInput")
    xeg_d = nc.dram_tensor("xeg", [P, NJ * SZJ], f8, kind="ExternalInput")
    xrl_d = nc.dram_tensor("xrl", [P, CK * N * S], f32, kind="ExternalInput")
    xrg_d = nc.dram_tensor("xrg", [P, CK * N * S], f32, kind="ExternalInput")
    # conv order: 0=Wk1(k_l) 1=Wv1(v_l) 2=Wq2*s(q_g) 3=Wq1*s(q_l) 4=Wk2(k_g) 5=Wv2(v_g)
    w3_d = nc.dram_tensor("w3", [6, P, NJ * SZW], f8, kind="ExternalInput")
    wp_d = nc.dram_tensor("wp", [2, P, 2 * 2 * C], f8, kind="ExternalInput")
    outl_d = nc.dram_tensor("out_l", [P, CK, N * S], f32, kind="ExternalOutput")
    outg_d = nc.dram_tensor("out_g", [P, CK, N * S], f32, kind="ExternalOutput")

    import bisect

    with tile.TileContext(nc) as tc:
        with (
            tc.tile_pool(name="singles", bufs=1) as singles,
            tc.tile_pool(name="wpool", bufs=3) as wpool,
            tc.tile_pool(name="attp", bufs=2) as attp,
            tc.tile_pool(name="small", bufs=8) as small,
            tc.tile_pool(name="ps", bufs=5, space="PSUM") as ps_pool,
            tc.tile_pool(name="psy", bufs=3, space="PSUM") as psy_pool,
        ):
            # ---- piece tiles: j-major blocks split for fine-grained DMA
            # dependencies (first matmul waits only for the first piece)
            def mk_xe_tiles(nm):
                return [singles.tile([P, b1 - b0, 2, N, S], f8,
                                     tag=f"xe_{nm}{k}", name=f"xe_{nm}{k}")
                        for k, (b0, b1) in enumerate(zip(PB[:-1], PB[1:]))]

            def mk_w_tiles():
                # per-piece tags so each conv's piece k reuses the slot of
                # the conv two back (bufs=2), giving DMA backpressure
                return [wpool.tile([P, b1 - b0, 2, C], f8, tag=f"w3_{k}",
                                   name=f"w3_{k}")
                        for k, (b0, b1) in enumerate(zip(PB[:-1], PB[1:]))]

            xg_t = mk_xe_tiles("g")
            xl_t = mk_xe_tiles("l")
            # allocation order == consumption order (slot reuse chain)
            wt = {cv: mk_w_tiles() for cv in (2, 4, 0, 3, 1, 5)}

            def dma_w(eng, cv, k):
                b0, b1 = PB[k], PB[k + 1]
                eng.dma_start(
                    wt[cv][k].rearrange("p a b c -> p (a b c)"),
                    w3_d[cv][:, b0 * SZW:b1 * SZW])

            def dma_x(eng, ts, dram, k):
                b0, b1 = PB[k], PB[k + 1]
                eng.dma_start(
                    ts[k].rearrange("p a b c d -> p (a b c d)"),
                    dram[:, b0 * SZJ:b1 * SZJ])

            # conv1's ~300GB/s piece stream is split across the two fast
            # queues (sync starts first and takes the tiny lead pieces);
            # later convs' weights are prefetched on whichever queue frees
            # up in consumption order. gpsimd (slow software queue) only
            # carries late-needed weights.
            NP = len(PB) - 1
            for k in range(NP):
                eng = nc.sync if k in (0, 1, 2, 4, 6) else nc.scalar
                dma_w(eng, 2, k)
                dma_x(eng, xg_t, xeg_d, k)
            for k in range(NP):
                dma_w(nc.sync, 4, k)
            for k in range(NP):
                dma_x(nc.sync, xl_t, xel_d, k)
            for cv in (0, 3):
                for k in range(NP):
                    dma_w(nc.scalar, cv, k)
            for cv in (1, 5):
                for k in range(NP):
                    dma_w(nc.gpsimd, cv, k)

            # wp fp8 pairs: [P, branch, pr, Ko, C] (ci chunk = 2*pr + Ko)
            wp_sb = singles.tile([P, 2, 2, 2, C], f8, tag="wp", name="wp")
            for b in range(2):
                nc.gpsimd.dma_start(
                    wp_sb[:, b].rearrange("p a b c -> p (a b c)"), wp_d[b])

            xr = {}
            for nm, tr in (("g", xrg_d), ("l", xrl_d)):
                xr[nm] = singles.tile([P, CK, N, S], f32, tag=f"xr_{nm}",
                                      name=f"xr_{nm}")
                nc.scalar.dma_start(
                    xr[nm].rearrange("p a b c -> p (a b c)"), tr[:, :])

            def pidx(j):
                return bisect.bisect_right(PB, j) - 1

            def xe_lhsT(xps, j, n):
                k = pidx(j)
                return xps[k][:, j - PB[k], :, n, :]

            def xe_rhs(xps, j):
                k = pidx(j)
                return xps[k][:, j - PB[k]]

            def wblk(cv, j):
                k = pidx(j)
                return wt[cv][k][:, j - PB[k]]

            def mm18(psum, xps, cv, n):
                """18 DoubleRow matmuls accumulating one x-stationary conv."""
                for j in range(NJ):
                    nc.tensor.matmul(
                        psum,
                        lhsT=xe_lhsT(xps, j, n),
                        rhs=wblk(cv, j),
                        start=(j == 0),
                        stop=(j == NJ - 1),
                        perf_mode=DR,
                    )

            def conv_first(xps, cv, dst):
                """First conv, j-outer over 4 live psums: compute starts
                after just the first x/w pieces have landed."""
                psums = [ps_pool.tile([P, C], f32, tag="ps", name="ps")
                         for _ in range(N)]
                for j in range(NJ):
                    for n in range(N):
                        nc.tensor.matmul(
                            psums[n],
                            lhsT=xe_lhsT(xps, j, n),
                            rhs=wblk(cv, j),
                            start=(j == 0),
                            stop=(j == NJ - 1),
                            perf_mode=DR,
                        )
                for n in range(N):
                    nc.vector.tensor_copy(out=dst[:, n, :], in_=psums[n])

            # deferred tensor-queue work: closures fire between conv MM
            # bursts so their upstream copies are long complete (no
            # head-of-line stalls in the in-order PE queue).
            pending = []

            def drain(k=1):
                for _ in range(k):
                    if pending:
                        pending.pop(0)()

            def conv_t(xps, cv, dst, push=None):
                """One x-stationary conv (all batches), transposed output."""
                for n in range(N):
                    psum = ps_pool.tile([P, C], f32, tag="ps", name="ps")
                    mm18(psum, xps, cv, n)
                    nc.vector.tensor_copy(out=dst[:, n, :], in_=psum)
                    drain()
                    if push is not None:
                        pending.append(push(n))

            def conv_v(xps, cv, vdst, drain_k=1):
                """V 3x3 conv, weight-stationary, normal [c, (n s)] output."""
                for co in range(CK):
                    psum = ps_pool.tile([P, C], f32, tag="ps", name="ps")
                    for j in range(NJ):
                        nc.tensor.matmul(
                            psum,
                            lhsT=wblk(cv, j)[:, :, co * P:(co + 1) * P],
                            rhs=xe_rhs(xps, j),
                            start=(j == 0),
                            stop=(j == NJ - 1),
                            perf_mode=DR,
                        )
                    nc.vector.tensor_copy(
                        out=vdst[:, co, :, 0:S],
                        in_=psum.rearrange("p (n s) -> p n s", n=N),
                    )
                    drain(drain_k)

            def qk_batch(q_sb, k_sb, n, pexp):
                """QK^T + exp for one batch into pexp [ck, cq] (bf16)."""
                for ck in range(CK):
                    psum = ps_pool.tile([P, C], f32, tag="ps", name="ps")
                    nc.tensor.matmul(
                        psum,
                        lhsT=k_sb[:, n, ck * P:(ck + 1) * P],
                        rhs=q_sb[:, n, :],
                        start=True,
                        stop=True,
                    )
                    # q,k carry a 32x factor each -> exp scale 1/1024
                    nc.scalar.activation(
                        out=pexp[:, ck, :], in_=psum, func=AF.Exp,
                        scale=1.0 / (WSCALE * WSCALE),
                    )

            def av_group(pexp, vaug, y_sb, cq, n):
                """One AV accumulation group (4 MMs + normalize), deferrable."""
                def run():
                    psy = psy_pool.tile([P, S + 1], f32, tag="psy",
                                        name="psy")
                    for ck in range(CK):
                        nc.tensor.matmul(
                            psy,
                            lhsT=pexp[:, ck, cq * P:(cq + 1) * P],
                            rhs=vaug[:, ck, n, :],
                            start=(ck == 0),
                            stop=(ck == CK - 1),
                        )
                    rec = small.tile([P, 1], f32, tag="rec", name="rec")
                    nc.vector.reciprocal(rec, psy[:, S:S + 1])
                    # ones col = 1 -> rec = 1/sum(exp); V carries 32x so
                    # y_sb holds 32*y, healthy for fp8. Normalize alternates
                    # scalar/vector so neither engine serializes the phase.
                    if (cq * N + n) % 2 == 0:
                        nc.scalar.activation(
                            out=y_sb[:, cq, n, :],
                            in_=psy[:, 0:S],
                            func=AF.Copy,
                            scale=rec,
                        )
                    else:
                        nc.vector.tensor_scalar_mul(
                            out=y_sb[:, cq, n, :],
                            in0=psy[:, 0:S],
                            scalar1=rec,
                        )
                return run

            def attend_av(pexps, vaug, y_sb, defer=False):
                for cq in range(CK):
                    for n in range(N):
                        g = av_group(pexps[n], vaug, y_sb, cq, n)
                        if defer:
                            pending.append(g)
                        else:
                            g()

            def proj_groups(y_sb, wp_idx, xres, out_d, out_tag):
                """1x1 conv (fp8 DoubleRow) + residual add + output DMA, as
                one deferrable closure per output chunk.
                psum = 2048*proj(y); xres is host-prescaled by 2048 and the
                host divides the gathered output by 2048 (exact, pow2)."""
                out_sb = singles.tile([P, CK, N, S], f32, tag=f"out_{out_tag}",
                                      name=f"out_{out_tag}")

                def co_group(co):
                    def run():
                        psum = ps_pool.tile([P, C], f32, tag="ps", name="ps")
                        for pr in range(2):
                            nc.tensor.matmul(
                                psum,
                                lhsT=wp_sb[:, wp_idx, pr, :,
                                           co * P:(co + 1) * P],
                                rhs=y_sb[:, 2 * pr:2 * pr + 2],
                                start=(pr == 0),
                                stop=(pr == 1),
                                perf_mode=DR,
                            )
                        nc.vector.tensor_add(
                            out=out_sb[:, co],
                            in0=psum.rearrange("p (n s) -> p n s", n=N),
                            in1=xres[:, co],
                        )
                        # two half-DMAs on separate queues drain the tail 2x
                        H2 = N * S // 2
                        nc.sync.dma_start(
                            out_d[:, co, 0:H2],
                            out_sb[:, co].rearrange("p a b -> p (a b)")[:, 0:H2])
                        nc.scalar.dma_start(
                            out_d[:, co, H2:2 * H2],
                            out_sb[:, co].rearrange(
                                "p a b -> p (a b)")[:, H2:2 * H2])
                    return run
                return [co_group(co) for co in range(CK)]

            # persistent attention operands ([s, n, c] bf16, 32x-scaled)
            k1 = singles.tile([P, N, C], bf16, tag="k1", name="k1")   # k_l
            q2 = singles.tile([P, N, C], bf16, tag="q2", name="q2")   # q_g
            q1 = singles.tile([P, N, C], bf16, tag="q1", name="q1")   # q_l
            k2 = singles.tile([P, N, C], bf16, tag="k2", name="k2")   # k_g
            vl = singles.tile([P, CK, N, S + 1], bf16, tag="vl", name="vl")
            vg = singles.tile([P, CK, N, S + 1], bf16, tag="vg", name="vg")
            # ones column = 1: denominator unscaled, so y_sb keeps V's 32x
            nc.vector.memset(vl[:, :, :, S:S + 1], 1.0)
            nc.vector.memset(vg[:, :, :, S:S + 1], 1.0)

            y_a = singles.tile([P, CK, N, S], f8, tag="y_a", name="y_a")
            y_b = singles.tile([P, CK, N, S], f8, tag="y_b", name="y_b")

            pexps_a = [attp.tile([P, CK, C], bf16, tag="pexp_a",
                                 name="pexp_a", bufs=4) for _ in range(N)]
            pexps_b = [attp.tile([P, CK, C], bf16, tag="pexp_b",
                                 name="pexp_b", bufs=4) for _ in range(N)]

            # branch A: global queries over local k/v -> out_g
            # branch B: local queries over global k/v -> out_l
            conv_first(xg_t, 2, q2)                             # q_g
            conv_t(xg_t, 4, k2)                                 # k_g
            conv_t(xl_t, 0, k1,                                 # k_l; QK-A
                   push=lambda n: (lambda: qk_batch(q2, k1, n, pexps_a[n])))
            conv_t(xl_t, 3, q1,                                 # q_l; QK-B
                   push=lambda n: (lambda: qk_batch(q1, k2, n, pexps_b[n])))
            conv_v(xl_t, 1, vl)                                 # v_l
            attend_av(pexps_a, vl, y_a, defer=True)             # AV-A deferred
            conv_v(xg_t, 5, vg, drain_k=4)                      # v_g + AV-A
            drain(len(pending))
            # AV-B with proj-A chunks interleaved to hide the normalize
            # latency ahead of proj-B
            avb = [av_group(pexps_b[n], vg, y_b, cq, n)
                   for cq in range(CK) for n in range(N)]
            proja = proj_groups(y_a, 1, xr["g"], outg_d, "g")
            projb = proj_groups(y_b, 0, xr["l"], outl_d, "l")
            for i, g in enumerate(avb):
                g()
                if i >= 9 and i % 2 == 1:
                    proja[(i - 9) // 2]()
            for g in projb:
                g()

    nc.finalize()
    return nc


def _prep_host_inputs(x_l, x_g, Wk1, Wq1, Wv1, Wk2, Wq2, Wv2, Wp1, Wp2,
                      resweight):
    """Build the 8 per-core input maps (numpy) from full inputs."""
    bf = ml_dtypes.bfloat16
    f8 = ml_dtypes.float8_e4m3
    scale = 1.0 / np.sqrt(np.float32(S))
    rw = np.float32(np.asarray(resweight))

    def t3(w, s):
        # [cout, cin, 3, 3] -> [9, cin, cout] fp32, scaled
        w = np.asarray(w, np.float32) * s
        return np.ascontiguousarray(w.transpose(2, 3, 1, 0).reshape(9, C, C))

    weffs = [
        t3(Wk1, WSCALE), t3(Wv1, WSCALE), t3(Wq2, scale * WSCALE),
        t3(Wq1, scale * WSCALE), t3(Wk2, WSCALE), t3(Wv2, WSCALE),
    ]
    # pairs per BLOCKS, j-major [6, P, NJ*2*C]
    w3 = np.empty((6, P, NJ, 2, C), np.float32)
    for cv, weff in enumerate(weffs):
        for j, (kind, a, t) in enumerate(BLOCKS):
            if kind == "t":
                w3[cv, :, j, 0] = weff[t, a * P:(a + 1) * P]
                w3[cv, :, j, 1] = weff[t + 1, a * P:(a + 1) * P]
            else:
                w3[cv, :, j, 0] = weff[t, a * P:(a + 1) * P]
                w3[cv, :, j, 1] = weff[t, (a + 1) * P:(a + 2) * P]
    w3 = np.clip(w3, -240, 240).astype(f8).reshape(6, P, NJ * 2 * C)

    # wp [2, P, 2(pr), 2(Ko), C] fp8: Wp.T * rw * 64 (y carries 32x; the
    # 32*64=2048 product scale is divided out on the host after gather)
    def tp(wpm):
        w = np.asarray(wpm, np.float32)[:, :, 0, 0].T * (rw * 64.0)
        return w.reshape(CK, P, C).transpose(1, 0, 2)        # [P, ck, cout]

    f8c = lambda a: np.clip(a, -240, 240).astype(f8)
    wp = f8c(np.stack([tp(Wp1), tp(Wp2)])).reshape(2, P, 2 * 2 * C)

    xl_p = np.pad(np.asarray(x_l, np.float32), ((0, 0), (0, 0), (1, 1), (1, 1)))
    xg_p = np.pad(np.asarray(x_g, np.float32), ((0, 0), (0, 0), (1, 1), (1, 1)))

    def mk_xe(xp, r0):
        # j-major [P, NJ*2*N*S] fp8: per j-block the pre-paired lhsT rows
        out = np.empty((P, CK, 9, N, S), np.float32)
        for t in range(9):
            ky, kx = t // 3, t % 3
            win = xp[:, :, r0 + ky:r0 + ky + ROWS, kx:kx + W]   # [N,C,4,32]
            out[:, :, t] = win.reshape(N, CK, P, S).transpose(2, 1, 0, 3)
        xj = np.empty((P, NJ, 2, N, S), np.float32)
        for j, (kind, a, t) in enumerate(BLOCKS):
            if kind == "t":
                xj[:, j, 0] = out[:, a, t]
                xj[:, j, 1] = out[:, a, t + 1]
            else:
                xj[:, j, 0] = out[:, a, t]
                xj[:, j, 1] = out[:, a + 1, t]
        return np.clip(xj, -240, 240).astype(f8).reshape(P, NJ * 2 * N * S)

    def mk_xr(x, r0):
        # [P, CK, N, S] f32 center rows (co-major), prescaled by 2048 to
        # match the 32*64 scale of the fp8 proj psum (host divides after)
        win = np.asarray(x, np.float32)[:, :, r0:r0 + ROWS, :] * 2048.0
        return np.ascontiguousarray(
            win.reshape(N, CK, P, S).transpose(2, 1, 0, 3)).reshape(P, -1)

    in_maps = []
    for core in range(N_CORES):
        r0 = core * ROWS
        in_maps.append({
            "xel": mk_xe(xl_p, r0),
            "xeg": mk_xe(xg_p, r0),
            "xrl": mk_xr(x_l, r0),
            "xrg": mk_xr(x_g, r0),
            "w3": w3,
            "wp": wp,
        })
    return in_maps


def kernel(x_l, x_g, Wk1, Wq1, Wv1, Wk2, Wq2, Wv2, Wp1, Wp2, resweight,
           _trace=False):
    from concourse.bass_utils import run_bass_kernel_spmd

    if "nc" not in _BUILT:
        _BUILT["nc"] = _build_bass()
    nc = _BUILT["nc"]

    in_maps = _prep_host_inputs(
        x_l, x_g, Wk1, Wq1, Wv1, Wk2, Wq2, Wv2, Wp1, Wp2, resweight
    )
    res = run_bass_kernel_spmd(
        nc, in_maps, core_ids=list(range(N_CORES)), trace=_trace
    )
    out_l = np.empty((N, C, H, W), np.float32)
    out_g = np.empty((N, C, H, W), np.float32)
    for core in range(N_CORES):
        r0 = core * ROWS
        for nm, full in (("out_l", out_l), ("out_g", out_g)):
            r = res.results[core][nm].reshape(P, CK, N, ROWS, W)
            # [p, co, n, r, c] -> full[n, co*128+p, r0+r, c]; /2048 undoes
            # the fp8 proj scale (exact power of two)
            full[:, :, r0:r0 + ROWS, :] = r.transpose(2, 1, 0, 3, 4).reshape(
                N, C, ROWS, W) * (1.0 / 2048.0)
    if _trace:
        kernel.last_result = res
    return out_l, out_g

